# revision 1
# baseline (speedup 1.0000x reference)
"""CenterLoss Trainium2 kernel.

reference semantics:
    feats  = features.reshape(4096, 96)
    label  = argmax(predicts.reshape(4096, 6625), axis=1)   # first occurrence
    d[n]   = ||feats[n] - centers[label[n]]||^2
    loss   = (sum_n clip(d[n], 1e-12, 1e12) + (4096*6625-4096)*1e-12) / 4096

Sharding: data-parallel over the flattened 4096-row batch axis, 512 rows per
core across 8 cores; centers replicated. Each core returns 128 per-partition
distance sums; the host does the final (tiny) reduction ("all-reduce").

Per-core pipeline (phases batched over the 4 row-tiles of 128 rows each so
engines never stall on each other's in-order queues; partition p holds rows
4p..4p+3 so the features DMA is one contiguous 1536B line per partition):
  A. stream predicts tiles [128, 6625] HBM->SBUF in two half-tile DMAs
     (~1.7 MB each, 13.3KB/partition descriptors; the memory-bound part,
     ~13.6 MB/core) + one features DMA
  B. per-half max-reduce over [128, chunks, 53] views -> cmax [128, 4*125];
     per-tile DVE max8 -> row max top8, max_index -> first-occurrence
     chunk id (uint32)
  C. indirect-DMA gather of each row's winning 53-wide chunk (212B/row,
     ~108KB/core extra HBM vs 543KB for 265-wide chunks); max8+max_index on
     the gathered chunk -> position; label = 53*chunk + pos; indirect-DMA
     gather of centers[label]
  D. acc[p] = sum over tiles/dims of (f - c)^2 via one gpsimd subtract +
     one ACT Square with accumulate
max8/max_index tie-breaking is first-occurrence, matching jnp.argmax
bit-exactly (f32 compares are exact).
"""

import numpy as np

NUM_CLASSES = 6625
FEAT_DIM = 96
N_ROWS = 4096           # B*T = 64*64
N_CORES = 8
ROWS_PER_CORE = N_ROWS // N_CORES   # 512
P = 128                 # partitions
N_TILES = ROWS_PER_CORE // P        # 4 row-tiles per core
CH = 53                 # chunk size (6625 = 125 * 53)
NCHUNK = NUM_CLASSES // CH          # 125
OUT_COLS = 1            # per-partition distance sums

_CACHE = {}


def _splits(n, k):
    """k split sizes of n, near-equal, larger first."""
    q, r = divmod(n, k)
    return [q + 1] * r + [q] * (k - r)


def _build_nc(reps=1, ablate="full", nsplit=2, rsplit=None, dual=0, dsub=0,
              ilv=1, xbufs=4, gat1=0, seng=0, pipe=0):
    if rsplit is None:
        rsplit = nsplit
    assert nsplit % rsplit == 0
    key = ("nc", reps, ablate, nsplit, rsplit, dual, dsub, ilv, xbufs, gat1,
           seng, pipe)
    if key in _CACHE:
        return _CACHE[key]

    from contextlib import ExitStack

    import concourse.bass as bass
    import concourse.tile as tile
    from concourse import bacc, mybir

    nc = bacc.Bacc(
        "TRN2",
        target_bir_lowering=False,
        debug=False,
        num_devices=N_CORES,
    )

    predicts = nc.dram_tensor(
        "predicts", [ROWS_PER_CORE, NUM_CLASSES], mybir.dt.float32,
        kind="ExternalInput",
    )
    features = nc.dram_tensor(
        "features", [ROWS_PER_CORE, FEAT_DIM], mybir.dt.float32,
        kind="ExternalInput",
    )
    centers = nc.dram_tensor(
        "centers", [NUM_CLASSES, FEAT_DIM], mybir.dt.float32,
        kind="ExternalInput",
    )
    out = nc.dram_tensor(
        "out", [P, OUT_COLS], mybir.dt.float32, kind="ExternalOutput",
    )

    fadd = mybir.AluOpType.add
    fmul = mybir.AluOpType.mult

    with tile.TileContext(nc) as tc:
        with ExitStack() as ctx:
            xpool = ctx.enter_context(tc.tile_pool(name="x", bufs=xbufs))
            small = ctx.enter_context(tc.tile_pool(name="small", bufs=5))
            const = ctx.enter_context(tc.tile_pool(name="const", bufs=1))

            # prebase[p, t] = rowid(p, t) * 125 : global chunk-row id base
            # rowid = t*128 + p normally; 4p + t with interleaved mapping
            pb4_i = const.tile([P, N_TILES], mybir.dt.int32)
            if ilv:
                nc.gpsimd.iota(
                    pb4_i[:], pattern=[[NCHUNK, N_TILES]], base=0,
                    channel_multiplier=N_TILES * NCHUNK)
            else:
                nc.gpsimd.iota(
                    pb4_i[:], pattern=[[P * NCHUNK, N_TILES]], base=0,
                    channel_multiplier=NCHUNK)
            pb4 = const.tile([P, N_TILES], mybir.dt.float32)
            nc.vector.tensor_copy(pb4[:], pb4_i[:])

            # per-partition distance accumulator (summed over row tiles)
            acc = const.tile([P, 1], mybir.dt.float32)

            # warm the ACT Square table set while DMAs stream
            actwarm = const.tile([P, 1], mybir.dt.float32)
            nc.scalar.activation(
                actwarm[:], pb4[:, 0:1],
                mybir.ActivationFunctionType.Square)

            # predicts viewed as rows of 53 elements: [512*125, 53]
            pred_chunks = predicts.ap().rearrange("r (a b) -> (r a) b", b=CH)
            if ilv:
                # partition p holds rows 4p..4p+3; tile t = rows {4p+t}
                pred_v = predicts.ap().rearrange(
                    "(p t) c -> p t c", t=N_TILES)
                feat_v = features.ap().rearrange(
                    "(p t) d -> p t d", t=N_TILES)
            else:
                pred_v = None
                feat_v = features.ap().rearrange("(t p) d -> p t d", p=P)

            # chunk-count split per partial DMA; reduces cover rsplit
            # groups of nsplit//rsplit DMA splits each
            QSPLIT = _splits(NCHUNK, nsplit)
            QOFF = [0]
            for q in QSPLIT:
                QOFF.append(QOFF[-1] + q)
            step = nsplit // rsplit
            ROFF = [QOFF[i * step] for i in range(rsplit)] + [NCHUNK]

            for _ in range(reps):
                # ---- phase A: stream predicts + features ----
                xs = []
                for t in range(N_TILES):
                    x = xpool.tile([P, NUM_CLASSES], mybir.dt.float32, tag="x")
                    for q in range(nsplit):
                        c0, c1 = QOFF[q] * CH, QOFF[q + 1] * CH
                        if seng:
                            eng = nc.gpsimd
                        else:
                            eng = nc.scalar if (dual and q % 2) else nc.sync
                        if ilv:
                            src = pred_v[:, t:t + 1, c0:c1]
                        else:
                            src = predicts.ap()[t * P:(t + 1) * P, c0:c1]
                        eng.dma_start(x[:, c0:c1], src)
                    xs.append(x)
                ftile = small.tile(
                    [P, N_TILES * FEAT_DIM], mybir.dt.float32, tag="feat")
                nc.sync.dma_start(ftile[:], feat_v)

                if ablate == "dma":
                    for t in range(N_TILES):
                        xv = xs[t][:].rearrange("p (a b) -> p a b", b=CH)
                        nc.vector.tensor_reduce(
                            acc[:, 0:1], xv[:, 0:1, :],
                            axis=mybir.AxisListType.XY, op=mybir.AluOpType.max)
                        nc.vector.tensor_reduce(
                            acc[:, 0:1], xv[:, NCHUNK - 1:NCHUNK, :],
                            axis=mybir.AxisListType.XY, op=mybir.AluOpType.max)
                    continue

                # ---- phase B: chunk maxes + per-tile top8/argmax chunk ----
                # pipe mode: each tile's L1 argmax + chunk gather issue right
                # after that tile's reduces, so the SWDGE gather latency
                # hides under the next tile's reduce work instead of
                # stalling the in-order DVE queue before phase C.
                cmax4 = small.tile(
                    [P, N_TILES * NCHUNK], mybir.dt.float32, tag="cmax4")
                top8 = small.tile(
                    [P, N_TILES * 8], mybir.dt.float32, tag="top8")
                cidx8 = small.tile(
                    [P, N_TILES * 8], mybir.dt.uint32, tag="cidx8")
                cidx_f = small.tile([P, N_TILES], mybir.dt.float32,
                                    tag="cidx_f")
                rsi4 = small.tile([P, N_TILES], mybir.dt.int32, tag="rsi4")
                chunkcat = small.tile(
                    [P, N_TILES * CH], mybir.dt.float32, tag="chunkcat")
                cidx8v = cidx8[:].rearrange("p (t e) -> p t e", e=8)

                def l1_tile(t):
                    nc.vector.max(
                        top8[:, t * 8:(t + 1) * 8],
                        cmax4[:, t * NCHUNK:(t + 1) * NCHUNK])
                    nc.vector.max_index(
                        cidx8[:, t * 8:(t + 1) * 8],
                        top8[:, t * 8:(t + 1) * 8],
                        cmax4[:, t * NCHUNK:(t + 1) * NCHUNK])
                    nc.vector.tensor_copy(
                        cidx_f[:, t:t + 1], cidx8v[:, t:t + 1, 0])
                    nc.vector.tensor_tensor(
                        out=rsi4[:, t:t + 1], in0=cidx_f[:, t:t + 1],
                        in1=pb4[:, t:t + 1], op=fadd)

                def gather_tile(t):
                    nc.gpsimd.indirect_dma_start(
                        out=chunkcat[:, t * CH:(t + 1) * CH],
                        out_offset=None,
                        in_=pred_chunks,
                        in_offset=bass.IndirectOffsetOnAxis(
                            ap=rsi4[:, t:t + 1], axis=0))

                for t in range(N_TILES):
                    xv = xs[t][:].rearrange("p (a b) -> p a b", b=CH)
                    for r in range(rsplit):
                        nc.vector.tensor_reduce(
                            cmax4[:, t * NCHUNK + ROFF[r]:
                                  t * NCHUNK + ROFF[r + 1]],
                            xv[:, ROFF[r]:ROFF[r + 1], :],
                            axis=mybir.AxisListType.X, op=mybir.AluOpType.max)
                    if pipe and ablate != "noidx":
                        l1_tile(t)
                        gather_tile(t)

                if ablate == "noidx":
                    nc.vector.tensor_copy(acc[:, 0:1], cmax4[:, 0:1])
                    continue

                if not pipe:
                    for t in range(N_TILES):
                        nc.vector.max(
                            top8[:, t * 8:(t + 1) * 8],
                            cmax4[:, t * NCHUNK:(t + 1) * NCHUNK])
                        nc.vector.max_index(
                            cidx8[:, t * 8:(t + 1) * 8],
                            top8[:, t * 8:(t + 1) * 8],
                            cmax4[:, t * NCHUNK:(t + 1) * NCHUNK])
                    nc.vector.tensor_copy(cidx_f[:], cidx8v[:, :, 0])
                    nc.vector.tensor_tensor(
                        out=rsi4[:], in0=cidx_f[:], in1=pb4[:], op=fadd)
                    for t in range(N_TILES):
                        gather_tile(t)

                # ---- phase C: position within chunk + centers gather ----
                ctop8 = small.tile(
                    [P, N_TILES * 8], mybir.dt.float32, tag="ctop8")
                pos8 = small.tile(
                    [P, N_TILES * 8], mybir.dt.uint32, tag="pos8")
                for t in range(N_TILES):
                    nc.vector.max(
                        ctop8[:, t * 8:(t + 1) * 8],
                        chunkcat[:, t * CH:(t + 1) * CH])
                    nc.vector.max_index(
                        pos8[:, t * 8:(t + 1) * 8],
                        ctop8[:, t * 8:(t + 1) * 8],
                        chunkcat[:, t * CH:(t + 1) * CH])

                pos_f = small.tile([P, N_TILES], mybir.dt.float32, tag="pos_f")
                nc.vector.tensor_copy(
                    pos_f[:],
                    pos8[:].rearrange("p (t e) -> p t e", e=8)[:, :, 0])

                # label = 53*chunk + pos
                labi4 = small.tile([P, N_TILES], mybir.dt.int32, tag="labi4")
                nc.vector.scalar_tensor_tensor(
                    out=labi4[:], in0=cidx_f[:], scalar=float(CH),
                    in1=pos_f[:], op0=fmul, op1=fadd)

                cselcat = small.tile(
                    [P, N_TILES * FEAT_DIM], mybir.dt.float32, tag="cselcat")
                if gat1:
                    nc.gpsimd.indirect_dma_start(
                        out=cselcat[:].rearrange(
                            "p (t d) -> p t d", d=FEAT_DIM),
                        out_offset=None,
                        in_=centers.ap(),
                        in_offset=bass.IndirectOffsetOnAxis(
                            ap=labi4[:, :], axis=0))
                else:
                    for t in range(N_TILES):
                        nc.gpsimd.indirect_dma_start(
                            out=cselcat[:, t * FEAT_DIM:(t + 1) * FEAT_DIM],
                            out_offset=None,
                            in_=centers.ap(),
                            in_offset=bass.IndirectOffsetOnAxis(
                                ap=labi4[:, t:t + 1], axis=0))

                # ---- phase D: acc[p] = sum_t sum_d (f - c)^2 ----
                diff = small.tile(
                    [P, N_TILES * FEAT_DIM], mybir.dt.float32, tag="diff")
                if dsub:
                    nc.vector.tensor_tensor(
                        out=diff[:], in0=ftile[:], in1=cselcat[:],
                        op=mybir.AluOpType.subtract)
                else:
                    nc.gpsimd.tensor_sub(diff[:], ftile[:], cselcat[:])
                sq = small.tile(
                    [P, N_TILES * FEAT_DIM], mybir.dt.float32, tag="sq")
                nc.scalar.activation(
                    sq[:], diff[:], mybir.ActivationFunctionType.Square,
                    accum_out=acc[:, 0:1])

            nc.sync.dma_start(out.ap()[:, :], acc[:])

    nc.compile()
    _CACHE[key] = nc
    return nc


def kernel(features, predicts, centers):
    from concourse.bass_utils import run_bass_kernel_spmd

    nc = _build_nc()

    feats = np.ascontiguousarray(
        np.asarray(features, dtype=np.float32).reshape(N_ROWS, FEAT_DIM))
    preds = np.ascontiguousarray(
        np.asarray(predicts, dtype=np.float32).reshape(N_ROWS, NUM_CLASSES))
    cents = np.ascontiguousarray(np.asarray(centers, dtype=np.float32))

    in_maps = []
    for m in range(N_CORES):
        s = slice(m * ROWS_PER_CORE, (m + 1) * ROWS_PER_CORE)
        in_maps.append({
            "predicts": np.ascontiguousarray(preds[s]),
            "features": np.ascontiguousarray(feats[s]),
            "centers": cents,
        })

    res = run_bass_kernel_spmd(nc, in_maps, core_ids=list(range(N_CORES)))

    d = np.concatenate([r["out"].reshape(-1) for r in res.results])
    d = np.clip(d.astype(np.float64), 1e-12, 1e12)
    total = d.sum() + (N_ROWS * NUM_CLASSES - N_ROWS) * 1e-12
    return np.asarray(total / N_ROWS, dtype=np.float32)

